# revision 28
# baseline (speedup 1.0000x reference)
"""Low-pass FFT filtering kernel for Trainium2 (8 NeuronCores).

Math: reference does, per (batch b, channel i), with X = x[b,:,:,i] (256x256):
    out_i = irfft(rfft(X, axis=0) * mask) + irfft(rfft(X, axis=1) * mask)
with mask keeping rfft modes 0..15 (ortho norm). That filter is an orthogonal
projection P = W @ W.T where W [256, 31] is the orthonormal basis
{1/sqrt(n), sqrt(2/n)cos(2pi k t/n), -sqrt(2/n)sin(2pi k t/n)}_{k=1..15}.
So  out_i = P @ X_i + X_i @ P = W @ (W.T @ X_i) + (X_i @ W) @ W.T.

Device schedule (per core = one batch, channel-major layouts):
  C = W.T @ Xcm   [31, I*N]   (Xcm = x[b] as [m, (i, n)])
  D = W.T @ Xt    [31, I*M]   (Xt  = x[b] as [n, (i, m)], host-transposed)
  out[m-tile, n'] per (i, j):  single K=63 matmul with
     lhsT = [Wt_j ; 0 ; D_i,j]  (63 x 128),  rhs = [C_i ; 0 ; Wt] (63 x 256)
  which accumulates both terms in one PSUM pass.
Inputs/weights are fp16 on device (PE runs fp16 at full rate vs 4x-cost
fp32 LOW_HIGH mode); accumulation is fp32 in PSUM and the output is fp32.
Sharding: batch b -> core b (8 cores, no communication).
"""

import os
import sys
import types

import numpy as np

import concourse.bass as bass
import concourse.bacc as bacc
import concourse.tile as tile
from concourse import mybir
from concourse.bass_utils import run_bass_kernel_spmd

B, M, N, I = 8, 256, 256, 32
KMAX = 16           # modes kept: 0..15
R = 2 * KMAX - 1    # 31 real basis vectors
FREE = I * N        # 8192
NCHUNK = 4          # channel chunks
CCOLS = FREE // NCHUNK   # 2048 cols = 8 channels per chunk
CH_PER_CHUNK = I // NCHUNK
F32 = mybir.dt.float32
F16 = mybir.dt.float16
NPDT = np.float16

LAST_RESULTS = None  # BassKernelResults of the most recent run (for test.py)


def _ensure_ntff_hook():
    """Provide antenv.axon_hooks if the image lacks it, so trace=True works."""
    try:
        from antenv.axon_hooks import get_axon_ntff_profile_hook  # noqa: F401
        return
    except ImportError:
        pass
    try:
        from trn_agent_boot.trn_boot import _ntff_profile_via_ctypes
        hook = _ntff_profile_via_ctypes("/opt/axon/libaxon_pjrt.so")
    except Exception:
        hook = None
    mod = types.ModuleType("antenv.axon_hooks")
    _state = {"hook": hook}
    mod.get_axon_ntff_profile_hook = lambda: _state["hook"]
    mod.set_axon_ntff_profile_hook = lambda h: _state.update(hook=h)
    sys.modules["antenv.axon_hooks"] = mod
    try:
        import antenv
        antenv.axon_hooks = mod
    except ImportError:
        pass


def _basis():
    t = np.arange(N)
    cols = [np.ones(N) / np.sqrt(N)]
    for k in range(1, KMAX):
        cols.append(np.sqrt(2.0 / N) * np.cos(2 * np.pi * k * t / N))
        cols.append(-np.sqrt(2.0 / N) * np.sin(2 * np.pi * k * t / N))
    return np.stack(cols, axis=1).astype(np.float32)  # [256, 31]


def _build_nc():
    nc = bacc.Bacc("TRN2", target_bir_lowering=False, debug=False,
                   enable_asserts=False, num_devices=8)

    xc = nc.declare_dram_parameter("xc", [M, FREE], F16, isOutput=False)
    xt = nc.declare_dram_parameter("xt", [N, I * M], F16, isOutput=False)
    w2 = nc.declare_dram_parameter("w2", [128, 2 * R], F16, isOutput=False)
    wz = nc.declare_dram_parameter("wz", [R + 1, CCOLS], F16, isOutput=False)
    zw = nc.declare_dram_parameter("zw", [R + 1, CCOLS], F16, isOutput=False)
    out = nc.declare_dram_parameter("out", [M, FREE], F16, isOutput=True)

    with tile.TileContext(nc) as tc:
        with (
            tc.tile_pool(name="const", bufs=1) as constp,
            tc.tile_pool(name="xin", bufs=4) as xin,
            tc.tile_pool(name="lr", bufs=3) as lrp,
            tc.tile_pool(name="oput", bufs=4) as outp,
            tc.tile_pool(name="pcd", bufs=4, space=bass.MemorySpace.PSUM) as pcdp,
            tc.tile_pool(name="p2", bufs=3, space=bass.MemorySpace.PSUM) as p2p,
            tc.tile_pool(name="jp", bufs=1, space=bass.MemorySpace.PSUM) as jpp,
        ):
            w2sb = constp.tile([128, 2 * R], F16)
            nc.sync.dma_start(out=w2sb[:], in_=w2[:])

            # PE warmup primer: ~96 tiny matmuls on a memset scratch tile.
            # They only depend on the memset, so they run during the input
            # DMA fill and flip the PE HAM clock gate to 2.4 GHz before the
            # real matmuls arrive.
            scratch = constp.tile([128, 64], F16)
            nc.gpsimd.memset(scratch[:], 0.0)
            jp = jpp.tile([31, 64], F32)
            for _ in range(96):
                nc.tensor.matmul(jp[:], scratch[:, 0:R], scratch[:, 0:64],
                                 start=True, stop=True)

            outs_todo = []
            for g in range(NCHUNK):
                gsl = slice(g * CCOLS, (g + 1) * CCOLS)

                x0 = xin.tile([128, CCOLS], F16, tag="x0")
                x1 = xin.tile([128, CCOLS], F16, tag="x1")
                t0 = xin.tile([128, CCOLS], F16, tag="t0")
                t1 = xin.tile([128, CCOLS], F16, tag="t1")
                # x rides the SP HWDGE ring, xt rides the ACT ring: the two
                # input streams transfer in parallel instead of FIFO-serial.
                # Chunk 0 interleaves across both rings so the first matmuls
                # start as early as possible.
                if g == 0:
                    nc.sync.dma_start(out=x0[:], in_=xc[0:128, gsl])
                    nc.scalar.dma_start(out=x1[:], in_=xc[128:256, gsl])
                    nc.sync.dma_start(out=t0[:], in_=xt[0:128, gsl])
                    nc.scalar.dma_start(out=t1[:], in_=xt[128:256, gsl])
                else:
                    nc.sync.dma_start(out=x0[:], in_=xc[0:128, gsl])
                    nc.sync.dma_start(out=x1[:], in_=xc[128:256, gsl])
                    nc.scalar.dma_start(out=t0[:], in_=xt[0:128, gsl])
                    nc.scalar.dma_start(out=t1[:], in_=xt[128:256, gsl])

                # L rows 0..30 = tiled W^T, row 31 = 0, rows 32..62 = D
                # R rows 0..30 = C, row 31 = 0, rows 32..62 = tiled W^T
                # const rows ride the GpSimd SWDGE ring so they never queue
                # behind either input stream
                Lg = lrp.tile([63, CCOLS], F16, tag="L")
                Rg = lrp.tile([63, CCOLS], F16, tag="R")
                nc.gpsimd.dma_start(out=Lg[0:32, :], in_=wz[:])
                nc.gpsimd.dma_start(out=Rg[31:63, :], in_=zw[:])

                for f in range(CCOLS // 512):
                    fsl = slice(f * 512, (f + 1) * 512)
                    # C into rows 0..30 (col group q0), D into rows 32..62
                    # (q32) of one PSUM bank: 4 dense back-to-back matmuls
                    pcd = pcdp.tile([63, 512], F32)
                    nc.tensor.matmul(pcd[0:R, :], w2sb[:, 0:R], x0[:, fsl],
                                     start=True, stop=False)
                    nc.tensor.matmul(pcd[0:R, :], w2sb[:, R:2 * R], x1[:, fsl],
                                     start=False, stop=True)
                    nc.tensor.matmul(pcd[32:63, :], w2sb[:, 0:R], t0[:, fsl],
                                     start=True, stop=False)
                    nc.tensor.matmul(pcd[32:63, :], w2sb[:, R:2 * R], t1[:, fsl],
                                     start=False, stop=True)
                    nc.scalar.copy(Rg[0:R, fsl], pcd[0:R, :])
                    nc.scalar.copy(Lg[32:63, fsl], pcd[32:63, :])

                o0 = outp.tile([128, CCOLS], F16, tag="o0")
                o1 = outp.tile([128, CCOLS], F16, tag="o1")
                # two channels share one full PSUM bank -> one DVE copy per pair
                for ip in range(CH_PER_CHUNK // 2):
                    for j, oj in enumerate((o0, o1)):
                        p2 = p2p.tile([128, 2 * N], F32)
                        for s in range(2):
                            il = 2 * ip + s
                            csl = slice(il * N, (il + 1) * N)
                            jsl = slice(il * N + j * 128, il * N + (j + 1) * 128)
                            nc.tensor.matmul(p2[:, s * N:(s + 1) * N],
                                             Lg[:, jsl], Rg[:, csl],
                                             start=True, stop=True)
                        nc.vector.tensor_copy(oj[:, 2 * ip * N:(2 * ip + 2) * N],
                                              p2[:])

                # output writes go at the tail of the SP ring: they sit behind
                # every input DMA in FIFO order, so input prefetch has strict
                # priority over output drain on HBM bandwidth
                outs_todo.append((slice(0, 128), gsl, o0))
                outs_todo.append((slice(128, 256), gsl, o1))

            for rsl, gsl2, oj in outs_todo:
                nc.sync.dma_start(out=out[rsl, gsl2], in_=oj[:])

    nc.finalize()
    return nc


_NC = None


def kernel(x: np.ndarray) -> np.ndarray:
    global _NC, LAST_RESULTS
    x = np.asarray(x)
    assert x.shape == (B, M, N, I), x.shape

    W = _basis().astype(NPDT)          # [256, 31]
    Wt = W.T.copy()                    # [31, 256]
    w2_np = np.concatenate([W[0:128, :], W[128:256, :]], axis=1)  # [128, 62]
    wtile = np.tile(Wt, (1, CH_PER_CHUNK))                        # [31, 2048]
    wz_np = np.concatenate([wtile, np.zeros((1, CCOLS), NPDT)], axis=0)
    zw_np = np.concatenate([np.zeros((1, CCOLS), NPDT), wtile], axis=0)

    if _NC is None:
        _NC = _build_nc()

    xq = np.asarray(x, dtype=NPDT)
    in_maps = []
    for b in range(B):
        xcm = np.ascontiguousarray(xq[b].transpose(0, 2, 1)).reshape(M, FREE)
        xtm = np.ascontiguousarray(xq[b].transpose(1, 2, 0)).reshape(N, I * M)
        in_maps.append({
            "xc": xcm, "xt": xtm,
            "w2": w2_np, "wz": wz_np, "zw": zw_np,
        })

    trace = bool(int(os.environ.get("KERNEL_TRACE", "0")))
    if trace:
        _ensure_ntff_hook()
    LAST_RESULTS = run_bass_kernel_spmd(_NC, in_maps, list(range(B)), trace=trace)

    out = np.empty((B, M, N, I), np.float32)
    for b in range(B):
        dev = LAST_RESULTS.results[b]["out"].astype(np.float32).reshape(M, I, N)
        out[b] = dev.transpose(0, 2, 1)
    return out
